# revision 38
# baseline (speedup 1.0000x reference)
"""HardTripletMiningLoss on 8 TRN2 NeuronCores (Bass, raw-block SPMD).

Math: with emb = concat(anchor, positive, negative) [N,D], labels = ind[:,0],
pd(a,b) = ||e_a - e_b||^2, the loss is the mean over triplets (i,j,k) of
td = pd(i,j) - pd(j,k) + A restricted to
  same(i,j) & ~same(j,k) & td > 0 & i != 0.
Only (i,j) pairs with same labels (and i>=1) contribute — ~N^2/L of N^2 pairs.
Each such pair p=(i,j) needs, over k: sum/count of relu(td), where
  td(p,k) = 2*g(j,k) - n_k + (n_i - 2*u_p + A),  u_p = <e_i, e_j>.

Device work per core (pairs sharded 8 ways, rows of [128, N] tiles):
  V[p,k] = g(j_p,k) - (n_k - mean_n)/2 - (BIG/2)*same(j_p,k)
via two accumulating bf16 matmuls per tile (emb^T gathered by j against
emb^T, then a one-hot label lhsT against an aux rhs whose columns carry
-(n_k - mean_n)/2 - BIG/2*onehot). The per-pair constant
  halfbias_p = (n_i - 2*u_p + A - mean_n)/2
is computed on HOST (O(pairs*D) numpy, same order as the gathers) so that
  relu(td) = 2*relu(V + halfbias_p)   and   td > 0  <=>  V > -halfbias_p.
ACT (scalar engine) does the relu row-sums, DVE (vector engine) the counts,
in parallel, one tile behind PE. Host sums the 8 cores' partials.
"""

import numpy as np
import ml_dtypes
from contextlib import ExitStack

import concourse.bass as bass
import concourse.mybir as mybir
from concourse.bass_utils import run_bass_kernel_spmd

F32 = mybir.dt.float32
BF16 = mybir.dt.bfloat16
AF = mybir.ActivationFunctionType
OP = mybir.AluOpType
NP_BF16 = ml_dtypes.bfloat16

N_CORES = 8
A_MARGIN = 0.2
BIG_HALF = 1.0e6  # BIG = 2e6 dominates any |td| (~1e3) by >>1e3x
PAD_NEG = -1.0e9  # halfbias for padding pair rows -> relu 0, count 0
MAX_TILES = 4     # per-core pair tiles per launch (PSUM bank budget)

_programs: dict = {}  # (T, N, L) -> bass.Bass
LAST_RES = None  # most recent BassKernelResults (for test harness tracing)


def _build_program(T: int, N: int, L: int) -> "bass.Bass":
    """One SPMD program: every core runs this with its own pair shard."""
    P = T * 128
    nc = bass.Bass()

    d_big = nc.declare_dram_parameter("big", [128, N + P], BF16, isOutput=False)
    d_aux = nc.declare_dram_parameter("aux", [L, P + N], BF16, isOutput=False)
    d_hcol = nc.declare_dram_parameter("hcol", [128, T], F32, isOutput=False)
    d_red = nc.declare_dram_parameter("red", [128, 2 * T], F32, isOutput=True)

    with ExitStack() as ctx:
        sb = lambda name, shape, dt: ctx.enter_context(nc.sbuf_tensor(name, shape, dt))
        ps = lambda name, shape: ctx.enter_context(nc.psum_tensor(name, shape, F32))

        # cols 0:N = emb^T (rhs of mm1), cols N:N+P = emb^T gathered by j (lhsT)
        big_sb = sb("big_sb", [128, N + P], BF16)
        # cols 0:P = one-hot(label_j) (lhsT of mm2), cols P:P+N = aux rhs
        aux_sb = sb("aux_sb", [L, P + N], BF16)
        hcol_sb = sb("hcol_sb", [128, T], F32)  # halfbias per tile column
        red_sb = sb("red_sb", [128, 2 * T], F32)    # relu sums | counts
        scr_a = [sb(f"scr_a{i}", [128, N], BF16) for i in range(2)]
        scr_d = sb("scr_d", [128, N], BF16)
        warm_sb = sb("warm_sb", [128, 1], BF16)
        psumV = [ps(f"psumV{t}", [128, N]) for t in range(T)]

        with (
            nc.semaphore("dma_in") as dma_in,
            nc.semaphore("auxs") as auxs,
            nc.semaphore("hc") as hc,
            nc.semaphore("mm") as mm,
            nc.semaphore("act") as act,
            nc.semaphore("cnt") as cnt,
            nc.semaphore("dma_out") as dma_out,
            nc.Block() as block,
        ):

            @block.sync
            def _(sync):
                # Three parallel DMA lanes: SP carries embt + hcol, the Act
                # queue carries embjt, Pool (SWDGE) carries aux.
                sync.dma_start(big_sb[:, 0:N], d_big[:, 0:N]).then_inc(dma_in, 16)
                sync.dma_start(hcol_sb[:], d_hcol[:]).then_inc(hc, 16)
                sync.wait_ge(cnt, T)
                sync.dma_start(d_red[:], red_sb[:]).then_inc(dma_out, 16)
                sync.wait_ge(dma_out, 16)

            @block.tensor
            def _(tensor):
                tensor.wait_ge(dma_in, 32)
                for t in range(T):
                    nc.tensor.matmul(
                        psumV[t][:], big_sb[:, N + 128 * t:N + 128 * (t + 1)],
                        big_sb[:, 0:N], start=True, stop=False,
                    )
                    if t == 0:
                        tensor.wait_ge(auxs, 16)
                    nc.tensor.matmul(
                        psumV[t][:], aux_sb[:, bass.ts(t, 128)],
                        aux_sb[:, P:P + N], start=False, stop=True,
                    ).then_inc(mm, 1)

            @block.scalar
            def _(scalar):
                scalar.dma_start(
                    big_sb[:, N:N + P], d_big[:, N:N + P]).then_inc(dma_in, 16)
                # warm the Relu table while the big DMAs are in flight
                scalar.wait_ge(hc, 16)
                nc.scalar.activation(warm_sb[:], hcol_sb[:, 0:1], AF.Relu)
                for t in range(T):
                    scalar.wait_ge(mm, t + 1)
                    if t >= 2:
                        # counter (Pool) must be done reading scr_a[t%2]
                        scalar.wait_ge(cnt, t - 1)
                    nc.scalar.activation(
                        scr_a[t % 2][:], psumV[t][:], AF.Relu,
                        bias=hcol_sb[:, t:t + 1],
                        accum_out=red_sb[:, t:t + 1],
                    ).then_inc(act, 1)

            @block.gpsimd
            def _(gpsimd):
                gpsimd.dma_start(aux_sb[:], d_aux[:]).then_inc(auxs, 16)

            @block.vector
            def _(vector):
                # count nonzeros of ACT's relu output (SBUF, not PSUM —
                # concurrent PSUM-bank access from two engines breaks HW)
                for t in range(T):
                    vector.wait_ge(act, t + 1)
                    if t:
                        vector.wait_ge(cnt, t)
                    nc.vector.tensor_scalar(
                        scr_d[:], scr_a[t % 2][:], 0.0,
                        None, OP.is_gt, OP.add,
                        accum_out=red_sb[:, T + t:T + t + 1],
                    ).then_inc(cnt, 1)

    return nc


def _get_program(T: int, N: int, L: int) -> "bass.Bass":
    key = (T, N, L)
    if key not in _programs:
        _programs[key] = _build_program(T, N, L)
    return _programs[key]


def _run_batch(shared, ii, jj, halfbias, T):
    """Run one SPMD launch over <=8*T*128 pairs; returns (sum, count) f64."""
    N, L, labels, emb_bf, embt_bf, auxr_bf = shared
    P = T * 128
    per = (len(ii) + N_CORES - 1) // N_CORES

    in_maps = []
    for c in range(N_CORES):
        sj = jj[c * per:(c + 1) * per]
        hb = halfbias[c * per:(c + 1) * per]
        m = len(sj)
        big = np.zeros((128, N + P), NP_BF16)
        big[:, :N] = embt_bf
        aux = np.zeros((L, P + N), NP_BF16)
        aux[:, P:] = auxr_bf
        hb_full = np.full(P, PAD_NEG, np.float32)
        if m:
            big[:, N:N + m] = emb_bf[sj].T
            aux[labels[sj], np.arange(m)] = 1.0
            hb_full[:m] = hb
        in_maps.append({
            "big": big,
            "aux": aux,
            "hcol": np.ascontiguousarray(hb_full.reshape(T, 128).T),
        })

    nc = _get_program(T, N, L)
    res = run_bass_kernel_spmd(nc, in_maps, list(range(N_CORES)))
    global LAST_RES
    LAST_RES = res
    s = 0.0
    cnt = 0.0
    for c in range(N_CORES):
        r = res.results[c]["red"].astype(np.float64)
        s += 2.0 * float(r[:, :T].sum())
        cnt += float(r[:, T:].sum())
    return s, cnt


def kernel(anchor, positive, negative, ind):
    anchor = np.asarray(anchor, dtype=np.float32)
    positive = np.asarray(positive, dtype=np.float32)
    negative = np.asarray(negative, dtype=np.float32)
    labels = np.asarray(ind).reshape(-1).astype(np.int64)

    emb = np.ascontiguousarray(np.concatenate([anchor, positive, negative], axis=0))
    N, D = emb.shape
    assert D == 128, f"kernel assumes D=128, got {D}"
    assert N == labels.shape[0]

    L = int(labels.max()) + 1 if labels.size else 1
    assert L <= 128, f"label ids must fit one-hot partitions, got {L}"

    # same-label (i, j) pairs, excluding the i=0 plane (keep[0] = False)
    same = labels[:, None] == labels[None, :]
    ii, jj = np.nonzero(same)
    sel = ii >= 1
    ii, jj = ii[sel].astype(np.int64), jj[sel].astype(np.int64)

    if len(ii) == 0:
        return np.float32(0.0)

    n = np.einsum("ij,ij->i", emb, emb, dtype=np.float64)
    mean_n = float(n.mean())
    u = np.einsum("ij,ij->i", emb[ii], emb[jj], dtype=np.float64)
    halfbias = ((n[ii] - 2.0 * u + A_MARGIN - mean_n) / 2.0).astype(np.float32)

    emb_bf = emb.astype(NP_BF16)
    embt_bf = np.ascontiguousarray(emb_bf.T)
    auxr = np.tile((-(n - mean_n) / 2.0).astype(np.float32), (L, 1))
    auxr[labels, np.arange(N)] -= BIG_HALF
    auxr_bf = auxr.astype(NP_BF16)
    shared = (N, L, labels, emb_bf, embt_bf, auxr_bf)

    batch_cap = N_CORES * MAX_TILES * 128
    s_tot, c_tot = 0.0, 0.0
    for b0 in range(0, len(ii), batch_cap):
        bi = ii[b0:b0 + batch_cap]
        bj = jj[b0:b0 + batch_cap]
        hb = halfbias[b0:b0 + batch_cap]
        per = (len(bi) + N_CORES - 1) // N_CORES
        T = max(1, (per + 127) // 128)
        s, c = _run_batch(shared, bi, bj, hb, T)
        s_tot += s
        c_tot += c

    if c_tot > 0:
        return np.float32(s_tot / max(c_tot, 1.0))
    return np.float32(0.0)


# revision 39
# speedup vs baseline: 1.0528x; 1.0528x over previous
"""HardTripletMiningLoss on 8 TRN2 NeuronCores (Bass, raw-block SPMD).

Math: with emb = concat(anchor, positive, negative) [N,D], labels = ind[:,0],
pd(a,b) = ||e_a - e_b||^2, the loss is the mean over triplets (i,j,k) of
td = pd(i,j) - pd(j,k) + A restricted to
  same(i,j) & ~same(j,k) & td > 0 & i != 0.
Only (i,j) pairs with same labels (and i>=1) contribute — ~N^2/L of N^2 pairs.
Each such pair p=(i,j) needs, over k: sum/count of relu(td), where
  td(p,k) = 2*g(j,k) - n_k + (n_i - 2*u_p + A),  u_p = <e_i, e_j>.

Device work per core (pairs sharded 8 ways, rows of [128, N] tiles):
  V[p,k] = g(j_p,k) - (n_k - mean_n)/2 - (BIG/2)*same(j_p,k)
via two accumulating bf16 matmuls per tile (emb^T gathered by j against
emb^T, then a one-hot label lhsT against an aux rhs whose columns carry
-(n_k - mean_n)/2 - BIG/2*onehot). The per-pair constant
  halfbias_p = (n_i - 2*u_p + A - mean_n)/2
is computed on HOST (O(pairs*D) numpy, same order as the gathers) so that
  relu(td) = 2*relu(V + halfbias_p)   and   td > 0  <=>  V > -halfbias_p.
ACT (scalar engine) does the relu row-sums, DVE (vector engine) the counts,
in parallel, one tile behind PE. Host sums the 8 cores' partials.
"""

import numpy as np
import ml_dtypes
from contextlib import ExitStack

import concourse.bass as bass
import concourse.mybir as mybir
from concourse.bass_utils import run_bass_kernel_spmd

F32 = mybir.dt.float32
BF16 = mybir.dt.bfloat16
AF = mybir.ActivationFunctionType
OP = mybir.AluOpType
NP_BF16 = ml_dtypes.bfloat16

N_CORES = 8
A_MARGIN = 0.2
BIG_HALF = 1.0e6  # BIG = 2e6 dominates any |td| (~1e3) by >>1e3x
PAD_NEG = -1.0e9  # halfbias for padding pair rows -> relu 0, count 0
MAX_TILES = 4     # per-core pair tiles per launch (PSUM bank budget)

_programs: dict = {}  # (T, N, L) -> bass.Bass
LAST_RES = None  # most recent BassKernelResults (for test harness tracing)


def _build_program(T: int, N: int, L: int) -> "bass.Bass":
    """One SPMD program: every core runs this with its own pair shard."""
    P = T * 128
    nc = bass.Bass()

    d_big = nc.declare_dram_parameter("big", [128, N + P], BF16, isOutput=False)
    d_aux = nc.declare_dram_parameter("aux", [L, P + N], BF16, isOutput=False)
    d_hcol = nc.declare_dram_parameter("hcol", [128, T], F32, isOutput=False)
    d_red = nc.declare_dram_parameter("red", [128, 2 * T], F32, isOutput=True)

    with ExitStack() as ctx:
        sb = lambda name, shape, dt: ctx.enter_context(nc.sbuf_tensor(name, shape, dt))
        ps = lambda name, shape: ctx.enter_context(nc.psum_tensor(name, shape, F32))

        # cols 0:N = emb^T (rhs of mm1), cols N:N+P = emb^T gathered by j (lhsT)
        big_sb = sb("big_sb", [128, N + P], BF16)
        # cols 0:P = one-hot(label_j) (lhsT of mm2), cols P:P+N = aux rhs
        aux_sb = sb("aux_sb", [L, P + N], BF16)
        hcol_sb = sb("hcol_sb", [128, T], F32)  # halfbias per tile column
        red_sb = sb("red_sb", [128, 2 * T], F32)    # relu sums | counts
        scr_a = [sb(f"scr_a{i}", [128, N], BF16) for i in range(2)]
        scr_d = sb("scr_d", [128, N], BF16)
        warm_sb = sb("warm_sb", [128, 1], BF16)
        psumV = [ps(f"psumV{t}", [128, N]) for t in range(T)]

        with (
            nc.semaphore("dma_in") as dma_in,
            nc.semaphore("auxs") as auxs,
            nc.semaphore("hc") as hc,
            nc.semaphore("mm") as mm,
            nc.semaphore("act") as act,
            nc.semaphore("cnt") as cnt,
            nc.semaphore("dma_out") as dma_out,
            nc.Block() as block,
        ):

            @block.sync
            def _(sync):
                # Three parallel DMA lanes: SP carries embt + hcol, the Act
                # queue carries embjt, Pool (SWDGE) carries aux.
                sync.dma_start(big_sb[:, 0:N], d_big[:, 0:N]).then_inc(dma_in, 16)
                sync.dma_start(hcol_sb[:], d_hcol[:]).then_inc(hc, 16)
                sync.wait_ge(cnt, T)
                # no wait on dma_out: the Block-exit drain covers the
                # in-flight output DMA, overlapping its ~1.9us completion
                # latency with teardown
                sync.dma_start(d_red[:], red_sb[:]).then_inc(dma_out, 16)

            @block.tensor
            def _(tensor):
                tensor.wait_ge(dma_in, 32)
                for t in range(T):
                    nc.tensor.matmul(
                        psumV[t][:], big_sb[:, N + 128 * t:N + 128 * (t + 1)],
                        big_sb[:, 0:N], start=True, stop=False,
                    )
                    if t == 0:
                        tensor.wait_ge(auxs, 16)
                    nc.tensor.matmul(
                        psumV[t][:], aux_sb[:, bass.ts(t, 128)],
                        aux_sb[:, P:P + N], start=False, stop=True,
                    ).then_inc(mm, 1)

            @block.scalar
            def _(scalar):
                scalar.dma_start(
                    big_sb[:, N:N + P], d_big[:, N:N + P]).then_inc(dma_in, 16)
                # warm the Relu table while the big DMAs are in flight
                scalar.wait_ge(hc, 16)
                nc.scalar.activation(warm_sb[:], hcol_sb[:, 0:1], AF.Relu)
                for t in range(T):
                    scalar.wait_ge(mm, t + 1)
                    if t >= 2:
                        # counter (Pool) must be done reading scr_a[t%2]
                        scalar.wait_ge(cnt, t - 1)
                    nc.scalar.activation(
                        scr_a[t % 2][:], psumV[t][:], AF.Relu,
                        bias=hcol_sb[:, t:t + 1],
                        accum_out=red_sb[:, t:t + 1],
                    ).then_inc(act, 1)

            @block.gpsimd
            def _(gpsimd):
                gpsimd.dma_start(aux_sb[:], d_aux[:]).then_inc(auxs, 16)

            @block.vector
            def _(vector):
                # count nonzeros of ACT's relu output (SBUF, not PSUM —
                # concurrent PSUM-bank access from two engines breaks HW)
                for t in range(T):
                    vector.wait_ge(act, t + 1)
                    if t:
                        vector.wait_ge(cnt, t)
                    nc.vector.tensor_scalar(
                        scr_d[:], scr_a[t % 2][:], 0.0,
                        None, OP.is_gt, OP.add,
                        accum_out=red_sb[:, T + t:T + t + 1],
                    ).then_inc(cnt, 1)

    return nc


def _get_program(T: int, N: int, L: int) -> "bass.Bass":
    key = (T, N, L)
    if key not in _programs:
        _programs[key] = _build_program(T, N, L)
    return _programs[key]


def _run_batch(shared, ii, jj, halfbias, T):
    """Run one SPMD launch over <=8*T*128 pairs; returns (sum, count) f64."""
    N, L, labels, emb_bf, embt_bf, auxr_bf = shared
    P = T * 128
    per = (len(ii) + N_CORES - 1) // N_CORES

    in_maps = []
    for c in range(N_CORES):
        sj = jj[c * per:(c + 1) * per]
        hb = halfbias[c * per:(c + 1) * per]
        m = len(sj)
        big = np.zeros((128, N + P), NP_BF16)
        big[:, :N] = embt_bf
        aux = np.zeros((L, P + N), NP_BF16)
        aux[:, P:] = auxr_bf
        hb_full = np.full(P, PAD_NEG, np.float32)
        if m:
            big[:, N:N + m] = emb_bf[sj].T
            aux[labels[sj], np.arange(m)] = 1.0
            hb_full[:m] = hb
        in_maps.append({
            "big": big,
            "aux": aux,
            "hcol": np.ascontiguousarray(hb_full.reshape(T, 128).T),
        })

    nc = _get_program(T, N, L)
    res = run_bass_kernel_spmd(nc, in_maps, list(range(N_CORES)))
    global LAST_RES
    LAST_RES = res
    s = 0.0
    cnt = 0.0
    for c in range(N_CORES):
        r = res.results[c]["red"].astype(np.float64)
        s += 2.0 * float(r[:, :T].sum())
        cnt += float(r[:, T:].sum())
    return s, cnt


def kernel(anchor, positive, negative, ind):
    anchor = np.asarray(anchor, dtype=np.float32)
    positive = np.asarray(positive, dtype=np.float32)
    negative = np.asarray(negative, dtype=np.float32)
    labels = np.asarray(ind).reshape(-1).astype(np.int64)

    emb = np.ascontiguousarray(np.concatenate([anchor, positive, negative], axis=0))
    N, D = emb.shape
    assert D == 128, f"kernel assumes D=128, got {D}"
    assert N == labels.shape[0]

    L = int(labels.max()) + 1 if labels.size else 1
    assert L <= 128, f"label ids must fit one-hot partitions, got {L}"

    # same-label (i, j) pairs, excluding the i=0 plane (keep[0] = False)
    same = labels[:, None] == labels[None, :]
    ii, jj = np.nonzero(same)
    sel = ii >= 1
    ii, jj = ii[sel].astype(np.int64), jj[sel].astype(np.int64)

    if len(ii) == 0:
        return np.float32(0.0)

    n = np.einsum("ij,ij->i", emb, emb, dtype=np.float64)
    mean_n = float(n.mean())
    u = np.einsum("ij,ij->i", emb[ii], emb[jj], dtype=np.float64)
    halfbias = ((n[ii] - 2.0 * u + A_MARGIN - mean_n) / 2.0).astype(np.float32)

    emb_bf = emb.astype(NP_BF16)
    embt_bf = np.ascontiguousarray(emb_bf.T)
    auxr = np.tile((-(n - mean_n) / 2.0).astype(np.float32), (L, 1))
    auxr[labels, np.arange(N)] -= BIG_HALF
    auxr_bf = auxr.astype(NP_BF16)
    shared = (N, L, labels, emb_bf, embt_bf, auxr_bf)

    batch_cap = N_CORES * MAX_TILES * 128
    s_tot, c_tot = 0.0, 0.0
    for b0 in range(0, len(ii), batch_cap):
        bi = ii[b0:b0 + batch_cap]
        bj = jj[b0:b0 + batch_cap]
        hb = halfbias[b0:b0 + batch_cap]
        per = (len(bi) + N_CORES - 1) // N_CORES
        T = max(1, (per + 127) // 128)
        s, c = _run_batch(shared, bi, bj, hb, T)
        s_tot += s
        c_tot += c

    if c_tot > 0:
        return np.float32(s_tot / max(c_tot, 1.0))
    return np.float32(0.0)
